# revision 15
# baseline (speedup 1.0000x reference)
"""Fused GEMM + bias + residual + AvgPool2d(2) + global-mean normalize, 8-core SPMD.

Reference computation (B=8192, IN_F=1024, OUT_F=4096, S=64, K=2):
    out_lin = x @ W.T + bias + y                  # (B, 4096)
    pooled  = avgpool2x2(out_lin.reshape(B,64,64))# (B, 32, 32)
    out     = pooled / pooled.mean()              # (B, 1, 32, 32)

Key algebraic folds (all exact):
  * The 2x2 avg-pool folds into the weight/bias/residual:
        pooled_raw[b, m] = x[b] . Wsum[m] + bias_sum[m] + y_sum[b, m]
    (m = 32*i + j pools OUT_F rows {128i+2j, 128i+2j+1, 128i+64+2j,
    128i+64+2j+1}; GEMM N-dim shrinks 4096 -> 1024).
  * The 1/4 pool factor cancels: out = pooled_raw * (B*1024 / gsum).
  * gsum = xsum_g . wcol_g + B*bias_tot + ytot_g.  The AllReduce payload
    carries the xsum and wcol VECTORS alongside the y-rowsum stats and the
    dot is computed after the AllReduce, so the AllReduce depends only on
    local data -- never on the AllGather or the GEMM.

Distribution: batch split 8 ways; W sharded by row-block (core c loads rows
[512c, 512c+512), 2 MiB), pools to its 128 features, AllGathers the bf16
pooled shard k-major (so the PE does no W transposes post-gather).

Schedule (v8).  Trace lessons from v2/v6/v7:
  * The ncfw mesh AllGather has a FIXED latency profile: ~8us handshake,
    then the local 7-peer broadcast drains at the fold_n-limited ~50GB/s
    regardless of model-DMA contention, then a recv/event tail.  A 256KB
    shard costs ~80us trigger-to-done.  Starving the model DMA queues to
    "help" it does nothing (v7: +55us).  The only lever is payload size:
    two half-shards pipeline their sends, so the FIRST half lands ~35us
    earlier and the second about where the single one did.
  * GEMM is split into k-halves A (k 0..511) and B (k 512..1023): G_A runs
    as soon as AllGather A lands, G_B (plus the bias row) when B lands.
    Each half's psum is added straight into ys by the DVE (no ACT drain,
    no bf16 bounce) -- psA's 4 banks recycle against the DVE adds.
  * The DVE queue order is pinned with wait_until stamps so the y-pool /
    stats / AllReduce-trigger chain NEVER sits behind a GEMM-gated add:
    pools+stats (default) < addA (0.11) < addB (0.12) < post-AR (0.135)
    < normalize (0.14).  The AllReduce readback is stamped 0.13 so it
    cannot head-of-line-block readback B on the scalar ring.
  * Rings: sync = wl half, y stream, AR payload, even stores.
    scalar = wl half, x0-7, readbacks A+B, AR readback, odd stores.
"""

import numpy as np

import concourse.bass as bass
import concourse.mybir as mybir
import concourse.tile as tile
from concourse import bacc
from concourse.bass import ts
from concourse.bass_utils import run_bass_kernel_spmd
from concourse.masks import make_identity

N_CORES = 8
B = 8192
BL = B // N_CORES          # 1024 batch rows per core
KF = 1024                  # IN_F (contraction)
NF = 4096                  # OUT_F
WL = NF // N_CORES         # 512 W rows per core
M = 1024                   # pooled features (32*32)
TOT = float(B * M)         # elements in the global mean
F32 = mybir.dt.float32
BF16 = mybir.dt.bfloat16
ADD = mybir.AluOpType.add
MULT = mybir.AluOpType.mult

_CACHE = {}


def build_nc():
    nc = bacc.Bacc("TRN2", target_bir_lowering=False, debug=False,
                   num_devices=N_CORES)
    x = nc.dram_tensor("x", [BL, KF], F32, kind="ExternalInput").ap()
    y = nc.dram_tensor("y", [BL, NF], F32, kind="ExternalInput").ap()
    w = nc.dram_tensor("w", [WL, KF], F32, kind="ExternalInput").ap()
    b = nc.dram_tensor("b", [1, NF], F32, kind="ExternalInput").ap()
    out = nc.dram_tensor("out", [BL, M], F32, kind="ExternalOutput").ap()

    # This core's W rows n = 128a + 64r + 2j + s pool to local feature
    # m_local = 32a + j; (r, s) are the pool taps.  j-major load keeps DMA
    # descriptors wide; partition p = 4j + a.
    w_pairs = w.rearrange("(n s) k -> n (s k)", s=2)          # [256, 2048]
    wv = w_pairs.rearrange("(a r j) kk -> r j a kk", a=4, r=2, j=32)

    with tile.TileContext(nc) as tc:
        with (
            tc.tile_pool(name="consts", bufs=1) as consts,
            tc.tile_pool(name="wload", bufs=1) as wload,
            tc.tile_pool(name="wtmp", bufs=1) as wtmp,
            tc.tile_pool(name="wtp", bufs=1) as wtp,
            tc.tile_pool(name="xload", bufs=8) as xload,
            tc.tile_pool(name="xtp", bufs=1) as xtp,
            tc.tile_pool(name="yload", bufs=4) as yload,
            tc.tile_pool(name="yup", bufs=3) as yup,
            tc.tile_pool(name="ysump", bufs=1) as ysump,
            tc.tile_pool(name="statsp", bufs=1) as statsp,
            tc.tile_pool(name="outp", bufs=3) as outp,
            tc.tile_pool(name="psA", bufs=4, space="PSUM") as psA,
            tc.tile_pool(name="psT", bufs=3, space="PSUM") as psT,
            tc.tile_pool(name="dram", bufs=1, space="DRAM") as dram,
        ):
            # ---- constants ----
            ident_f = consts.tile([128, 128], F32)
            make_identity(nc, ident_f)
            ident_b = consts.tile([128, 128], BF16)
            make_identity(nc, ident_b)
            ones_row = consts.tile([1, 128], BF16)
            nc.vector.memset(ones_row, 1.0)
            ones_col_f = consts.tile([128, 1], F32)
            nc.vector.memset(ones_col_f, 1.0)
            ones_row_f = consts.tile([1, 128], F32)
            nc.vector.memset(ones_row_f, 1.0)

            # ---- queue heads ----
            wl = wload.tile([128, 2, 2048], F32, tag="wl", name="wl")
            nc.scalar.dma_start(out=wl[:, 0, :], in_=wv[0])
            nc.sync.dma_start(out=wl[:, 1, :], in_=wv[1])
            wlv = wl.rearrange("p r (s k) -> p r s k", s=2)

            xfs = []
            for bt in range(8):
                xf = xload.tile([128, KF], F32, tag="xf", name=f"xf{bt}")
                nc.scalar.dma_start(out=xf, in_=x[ts(bt, 128), :])
                xfs.append(xf)

            yts = {}

            def load_y(bt):
                for nh in range(2):
                    yt = yload.tile([128, 2048], F32, tag="yt",
                                    name=f"yt{bt}_{nh}")
                    nc.sync.dma_start(out=yt, in_=y[ts(bt, 128), ts(nh, 2048)])
                    yts[(bt, nh)] = yt

            load_y(0)
            load_y(1)

            # ---- stats: the AllReduce payload.  cols 0-15 y-pool rowsums,
            # 16-23 xsum (per k), 24-31 wcol partial (per k). ----
            stats = statsp.tile([128, 32], F32)

            # ---- W pooling + k-major transpose + 2 half AllGathers ----
            t1 = wtmp.tile([128, KF], F32)
            t2 = wtmp.tile([128, KF], F32)
            wsum = wtmp.tile([128, KF], BF16)
            wtc = wtmp.tile([128, 8, 128], BF16)
            nc.vector.tensor_add(t1, wlv[:, 0, 0], wlv[:, 0, 1])
            nc.vector.tensor_add(t2, wlv[:, 1, 0], wlv[:, 1, 1])
            nc.vector.tensor_add(wsum, t1, t2)
            for kb in range(8):
                ptw = psT.tile([128, 128], BF16, tag="pt", name=f"ptw{kb}")
                nc.tensor.transpose(ptw, wsum[:, ts(kb, 128)], ident_b)
                # undo the j-major load permutation p=4j+a -> 32a+j
                nc.vector.tensor_copy(
                    out=wtc[:, kb, :].rearrange("k (a j) -> k j a", a=4),
                    in_=ptw.rearrange("k (j a) -> k j a", a=4))
            # local wcol partial: sum over this core's 128 pooled features
            nc.vector.reduce_sum(
                out=stats[:, 24:32].rearrange("p (n o) -> p n o", o=1),
                in_=wtc, axis=mybir.AxisListType.X)

            cc_in = {}
            cc_out = {}
            for h in range(2):
                cc_in[h] = dram.tile([128, 512], BF16, space="DRAM",
                                     name=f"cc_w_in{h}")
                cc_out[h] = dram.tile([N_CORES * 128, 512], BF16,
                                      space="DRAM", name=f"cc_w_out{h}")
                nc.gpsimd.dma_start(out=cc_in[h], in_=wtc[:, ts(h, 4), :])
                nc.gpsimd.collective_compute(
                    "AllGather", mybir.AluOpType.bypass,
                    replica_groups=[list(range(N_CORES))],
                    ins=[cc_in[h].opt()], outs=[cc_out[h].opt()])

            # bias load (SWDGE, reuses the W slot once pooled)
            bload = wload.tile([1, NF], F32, tag="wl", name="bload")
            nc.gpsimd.dma_start(out=bload, in_=b)

            # ---- helpers ----
            ys_tiles = {}
            xT_tiles = {}

            def pool_tile(bt):
                ys = ys_tiles[bt]
                for nh in range(2):
                    yt = yts.pop((bt, nh))
                    ytv = yt.rearrange("p (q s) -> p q s", s=2)
                    u = yup.tile([128, KF], F32, tag="u", name=f"u{bt}_{nh}")
                    ueng = nc.vector if nh == 0 else nc.gpsimd
                    ueng.tensor_add(u, ytv[:, :, 0], ytv[:, :, 1])
                    u2 = u.rearrange("p (i r j) -> p i r j", r=2, j=32)
                    c = 2 * bt + nh
                    nc.vector.tensor_add(
                        ys[:, ts(nh, 512)].rearrange("p (i j) -> p i j", j=32),
                        u2[:, :, 0, :], u2[:, :, 1, :])
                    nc.vector.reduce_sum(
                        out=stats[:, c:c + 1],
                        in_=ys[:, ts(nh, 512)], axis=mybir.AxisListType.X)

            def xtrans(bt):
                xT = xtp.tile([128, 8, 128], BF16, tag=f"xT{bt}",
                              name=f"xT{bt}")
                for kb in range(8):
                    pt = psT.tile([128, 128], F32, tag="pt",
                                  name=f"ptx{bt}_{kb}")
                    nc.tensor.transpose(pt, xfs[bt][:, ts(kb, 128)], ident_f)
                    nc.scalar.copy(out=xT[:, kb, :], in_=pt)
                xT_tiles[bt] = xT

            def xsred(k):
                xs_r = statsp.tile([128, 8, 1], F32, tag="xs_r", bufs=2,
                                   name=f"xs_r{k}")
                nc.vector.reduce_sum(out=xs_r, in_=xT_tiles[k],
                                     axis=mybir.AxisListType.X)
                if k == 0:
                    nc.vector.tensor_copy(out=stats[:, 16:24],
                                          in_=xs_r[:, :, 0])
                else:
                    nc.vector.tensor_add(stats[:, 16:24], stats[:, 16:24],
                                         xs_r[:, :, 0])

            for bt in range(8):
                ys_tiles[bt] = ysump.tile([128, M], F32, tag=f"ys{bt}",
                                          name=f"ys{bt}")

            # ---- bias pooling (feeds the G_B bias matmul + the payload) ----
            bsum = consts.tile([1, 32, 32], F32)
            bsum_bf = consts.tile([1, M], BF16)
            btot = consts.tile([1, 1], F32)
            btot_s = consts.tile([1, 1], F32)
            blv = bload.rearrange("o (i r j s) -> o i r j s", r=2, j=32, s=2)
            nc.vector.tensor_add(bsum, blv[:, :, 0, :, 0], blv[:, :, 0, :, 1])
            nc.vector.tensor_add(bsum, bsum, blv[:, :, 1, :, 0])
            nc.vector.tensor_add(bsum, bsum, blv[:, :, 1, :, 1])
            nc.vector.tensor_copy(
                out=bsum_bf, in_=bsum.rearrange("o i j -> o (i j)"))
            nc.vector.reduce_sum(
                out=btot, in_=bsum.rearrange("o i j -> o (i j)"),
                axis=mybir.AxisListType.X)
            nc.vector.tensor_scalar_mul(btot_s, btot, float(BL))

            # ---- gathered W readbacks (scalar ring; B after A) ----
            wt8 = {0: wtp.tile([128, 8, 4, 128], BF16, name="wt8a"),
                   1: wtp.tile([128, 8, 4, 128], BF16, name="wt8b")}
            for h in range(2):
                for r in range(8):
                    nc.scalar.dma_start(out=wt8[h][:, r],
                                        in_=cc_out[h][ts(r, 128), :])

            # ---- main loop: y stream + pooling + x transposes + stats ----
            for bt in range(8):
                if bt >= 2:
                    load_y(bt)
                pool_tile(bt)
                xtrans(bt)
                xsred(bt)

            # fold the bias moment into the payload (after all col-0 writes)
            nc.vector.tensor_add(stats[0:1, 0:1], stats[0:1, 0:1], btot_s)

            # ---- AllReduce: fires at y-stream end, AllGather-independent ----
            cc_ar_in = dram.tile([128, 32], F32, space="DRAM")
            cc_ar_out = dram.tile([128, 32], F32, space="DRAM")
            nc.sync.dma_start(out=cc_ar_in, in_=stats)
            nc.gpsimd.collective_compute(
                "AllReduce", ADD,
                replica_groups=[list(range(N_CORES))],
                ins=[cc_ar_in.opt()], outs=[cc_ar_out.opt()])

            # ---- GEMM halves: psum added straight into ys by the DVE.
            # Stamps keep these AFTER the pool/stats chain in the DVE queue
            # so a late AllGather can never stall the AllReduce path. ----
            def gemm_half(h, stamp):
                with tc.tile_wait_until(stamp):
                    for bt in range(8):
                        for mh in range(2):
                            mm = psA.tile([128, 512], F32, tag="mm",
                                          name=f"mm{h}_{bt}_{mh}")
                            for kb in range(4):
                                nc.tensor.matmul(
                                    mm, xT_tiles[bt][:, 4 * h + kb, :],
                                    wt8[h][:, 4 * mh:4 * mh + 4, kb, :],
                                    start=(kb == 0),
                                    stop=(h == 0 and kb == 3))
                            if h == 1:
                                nc.tensor.matmul(mm, ones_row,
                                                 bsum_bf[:, ts(mh, 512)],
                                                 start=False, stop=True)
                            nc.vector.tensor_add(ys_tiles[bt][:, ts(mh, 512)],
                                                 ys_tiles[bt][:, ts(mh, 512)],
                                                 mm)

            gemm_half(0, 0.11)
            gemm_half(1, 0.12)

            # ---- post-AR: gsum = sum(stats cols 0-15) + xsum_g . wcol_g ----
            with tc.tile_wait_until(0.13):
                S = statsp.tile([128, 32], F32)
                nc.scalar.dma_start(out=S, in_=cc_ar_out)
            with tc.tile_wait_until(0.135):
                tmp8 = statsp.tile([128, 8], F32)
                nc.vector.tensor_mul(tmp8, S[:, 16:24], S[:, 24:32])
                red1 = statsp.tile([128, 1], F32)
                red2 = statsp.tile([128, 1], F32)
                nc.vector.reduce_sum(out=red1, in_=S[:, 0:16],
                                     axis=mybir.AxisListType.X)
                nc.vector.reduce_sum(out=red2, in_=tmp8,
                                     axis=mybir.AxisListType.X)
                pcol2 = statsp.tile([128, 1], F32)
                nc.vector.tensor_add(pcol2, red1, red2)
                # partition-sum + broadcast on the (idle) PE
                g1ps = psT.tile([128, 128], F32, tag="pt", name="g1ps")
                nc.tensor.matmul(g1ps[0:1, 0:1], ones_col_f, pcol2,
                                 start=True, stop=True)
                g1sb = statsp.tile([1, 1], F32)
                nc.scalar.copy(out=g1sb, in_=g1ps[0:1, 0:1])
                gbps = psT.tile([128, 128], F32, tag="pt", name="gbps")
                nc.tensor.matmul(gbps[:, 0:1], ones_row_f, g1sb,
                                 start=True, stop=True)
                rsb = statsp.tile([128, 1], F32)
                nc.vector.reciprocal(rsb, gbps[:, 0:1])

            # ---- normalize + store, per-tile pipelined ----
            with tc.tile_wait_until(0.14):
                for bt in range(8):
                    meng = nc.vector if bt % 2 == 0 else nc.gpsimd
                    ot = outp.tile([128, M], F32, tag="ot", name=f"ot{bt}")
                    meng.tensor_scalar(out=ot, in0=ys_tiles[bt],
                                       scalar1=rsb, scalar2=TOT,
                                       op0=MULT, op1=MULT)
                    reng = nc.sync if bt % 2 == 0 else nc.scalar
                    reng.dma_start(out=out[ts(bt, 128), :], in_=ot)

    nc.compile()
    return nc


def _run(inputs, trace=False):
    if "nc" not in _CACHE:
        _CACHE["nc"] = build_nc()
    nc = _CACHE["nc"]
    x = np.ascontiguousarray(np.asarray(inputs["x"], dtype=np.float32))
    y = np.ascontiguousarray(np.asarray(inputs["y"], dtype=np.float32))
    w = np.ascontiguousarray(np.asarray(inputs["weight"], dtype=np.float32))
    b = np.ascontiguousarray(
        np.asarray(inputs["bias"], dtype=np.float32).reshape(1, NF))
    in_maps = [
        {"x": x[c * BL:(c + 1) * BL], "y": y[c * BL:(c + 1) * BL],
         "w": np.ascontiguousarray(w[c * WL:(c + 1) * WL]), "b": b}
        for c in range(N_CORES)
    ]
    res = run_bass_kernel_spmd(nc, in_maps, core_ids=list(range(N_CORES)),
                               trace=trace)
    full = np.concatenate([res.results[c]["out"] for c in range(N_CORES)],
                          axis=0)
    return full.reshape(B, 1, 32, 32), res


def kernel(**inputs) -> np.ndarray:
    out, _ = _run(inputs, trace=False)
    return out


# revision 16
# speedup vs baseline: 1.3697x; 1.3697x over previous
"""Fused GEMM + bias + residual + AvgPool2d(2) + global-mean normalize, 8-core SPMD.

Reference computation (B=8192, IN_F=1024, OUT_F=4096, S=64, K=2):
    out_lin = x @ W.T + bias + y                  # (B, 4096)
    pooled  = avgpool2x2(out_lin.reshape(B,64,64))# (B, 32, 32)
    out     = pooled / pooled.mean()              # (B, 1, 32, 32)

Key algebraic folds (all exact):
  * The 2x2 avg-pool folds into the weight/bias/residual:
        pooled_raw[b, m] = x[b] . Wsum[m] + bias_sum[m] + y_sum[b, m]
    (m = 32*i + j pools OUT_F rows {128i+2j, 128i+2j+1, 128i+64+2j,
    128i+64+2j+1}; GEMM N-dim shrinks 4096 -> 1024).
  * The 1/4 pool factor cancels: out = pooled_raw * (B*1024 / gsum).
  * gsum = xsum_g . wcol_g + B*bias_tot + ytot_g.  The AllReduce payload
    carries the xsum and wcol VECTORS alongside the y-rowsum stats and the
    dot is computed after the AllReduce, so the AllReduce depends only on
    local data -- never on the AllGather or the GEMM.

Distribution: batch split 8 ways; W sharded by row-block (core c loads rows
[512c, 512c+512), 2 MiB), pools to its 128 features, AllGathers the bf16
pooled shard k-major (so the PE does no W transposes post-gather).

Schedule (v8).  Trace lessons from v2/v6/v7:
  * The ncfw mesh AllGather has a FIXED latency profile: ~8us handshake,
    then the local 7-peer broadcast drains at the fold_n-limited ~50GB/s
    regardless of model-DMA contention, then a recv/event tail.  A 256KB
    shard costs ~80us trigger-to-done.  Starving the model DMA queues to
    "help" it does nothing (v7: +55us).  The only lever is payload size:
    two half-shards pipeline their sends, so the FIRST half lands ~35us
    earlier and the second about where the single one did.
  * GEMM is split into k-halves A (k 0..511) and B (k 512..1023): G_A runs
    as soon as AllGather A lands, G_B (plus the bias row) when B lands.
    Each half's psum is added straight into ys by the DVE (no ACT drain,
    no bf16 bounce) -- psA's 4 banks recycle against the DVE adds.
  * The DVE queue order is pinned with wait_until stamps so the y-pool /
    stats / AllReduce-trigger chain NEVER sits behind a GEMM-gated add:
    pools+stats (default) < addA (0.11) < addB (0.12) < post-AR (0.135)
    < normalize (0.14).  The AllReduce readback is stamped 0.13 so it
    cannot head-of-line-block readback B on the scalar ring.
  * Rings: sync = wl half, y stream, AR payload, even stores.
    scalar = wl half, x0-7, readbacks A+B, AR readback, odd stores.
"""

import numpy as np

import concourse.bass as bass
import concourse.mybir as mybir
import concourse.tile as tile
from concourse import bacc
from concourse.bass import ts
from concourse.bass_utils import run_bass_kernel_spmd
from concourse.masks import make_identity

N_CORES = 8
B = 8192
BL = B // N_CORES          # 1024 batch rows per core
KF = 1024                  # IN_F (contraction)
NF = 4096                  # OUT_F
WL = NF // N_CORES         # 512 W rows per core
M = 1024                   # pooled features (32*32)
TOT = float(B * M)         # elements in the global mean
F32 = mybir.dt.float32
BF16 = mybir.dt.bfloat16
ADD = mybir.AluOpType.add
MULT = mybir.AluOpType.mult

_CACHE = {}


def build_nc():
    nc = bacc.Bacc("TRN2", target_bir_lowering=False, debug=False,
                   num_devices=N_CORES)
    x = nc.dram_tensor("x", [BL, KF], F32, kind="ExternalInput").ap()
    y = nc.dram_tensor("y", [BL, NF], F32, kind="ExternalInput").ap()
    w = nc.dram_tensor("w", [WL, KF], F32, kind="ExternalInput").ap()
    b = nc.dram_tensor("b", [1, NF], F32, kind="ExternalInput").ap()
    out = nc.dram_tensor("out", [BL, M], F32, kind="ExternalOutput").ap()

    # This core's W rows n = 128a + 64r + 2j + s pool to local feature
    # m_local = 32a + j; (r, s) are the pool taps.  j-major load keeps DMA
    # descriptors wide; partition p = 4j + a.
    w_pairs = w.rearrange("(n s) k -> n (s k)", s=2)          # [256, 2048]
    wv = w_pairs.rearrange("(a r j) kk -> r j a kk", a=4, r=2, j=32)

    with tile.TileContext(nc) as tc:
        with (
            tc.tile_pool(name="consts", bufs=1) as consts,
            tc.tile_pool(name="wload", bufs=1) as wload,
            tc.tile_pool(name="wtmp", bufs=1) as wtmp,
            tc.tile_pool(name="wtp", bufs=1) as wtp,
            tc.tile_pool(name="xload", bufs=8) as xload,
            tc.tile_pool(name="xtp", bufs=1) as xtp,
            tc.tile_pool(name="yload", bufs=4) as yload,
            tc.tile_pool(name="yup", bufs=3) as yup,
            tc.tile_pool(name="ysump", bufs=1) as ysump,
            tc.tile_pool(name="statsp", bufs=1) as statsp,
            tc.tile_pool(name="outp", bufs=3) as outp,
            tc.tile_pool(name="psA", bufs=4, space="PSUM") as psA,
            tc.tile_pool(name="psT", bufs=3, space="PSUM") as psT,
            tc.tile_pool(name="dram", bufs=1, space="DRAM") as dram,
        ):
            # ---- constants ----
            ident_f = consts.tile([128, 128], F32)
            make_identity(nc, ident_f)
            ident_b = consts.tile([128, 128], BF16)
            make_identity(nc, ident_b)
            ones_row = consts.tile([1, 128], BF16)
            nc.vector.memset(ones_row, 1.0)
            ones_col_f = consts.tile([128, 1], F32)
            nc.vector.memset(ones_col_f, 1.0)
            ones_row_f = consts.tile([1, 128], F32)
            nc.vector.memset(ones_row_f, 1.0)

            # ---- queue heads ----
            wl = wload.tile([128, 2, 2048], F32, tag="wl", name="wl")
            nc.scalar.dma_start(out=wl[:, 0, :], in_=wv[0])
            nc.sync.dma_start(out=wl[:, 1, :], in_=wv[1])
            wlv = wl.rearrange("p r (s k) -> p r s k", s=2)

            xfs = []
            for bt in range(8):
                xf = xload.tile([128, KF], F32, tag="xf", name=f"xf{bt}")
                nc.scalar.dma_start(out=xf, in_=x[ts(bt, 128), :])
                xfs.append(xf)

            yts = {}

            def load_y(bt):
                for nh in range(2):
                    yt = yload.tile([128, 2048], F32, tag="yt",
                                    name=f"yt{bt}_{nh}")
                    nc.sync.dma_start(out=yt, in_=y[ts(bt, 128), ts(nh, 2048)])
                    yts[(bt, nh)] = yt

            load_y(0)
            load_y(1)

            # ---- stats: the AllReduce payload.  cols 0-15 y-pool rowsums,
            # 16-23 xsum (per k), 24-31 wcol partial (per k). ----
            stats = statsp.tile([128, 32], F32)

            # ---- W pooling + k-major transpose + 2 half AllGathers ----
            t1 = wtmp.tile([128, KF], F32)
            t2 = wtmp.tile([128, KF], F32)
            wsum = wtmp.tile([128, KF], BF16)
            wtc = wtmp.tile([128, 8, 128], BF16)
            nc.vector.tensor_add(t1, wlv[:, 0, 0], wlv[:, 0, 1])
            nc.vector.tensor_add(t2, wlv[:, 1, 0], wlv[:, 1, 1])
            nc.vector.tensor_add(wsum, t1, t2)
            for kb in range(8):
                ptw = psT.tile([128, 128], BF16, tag="pt", name=f"ptw{kb}")
                nc.tensor.transpose(ptw, wsum[:, ts(kb, 128)], ident_b)
                # undo the j-major load permutation p=4j+a -> 32a+j
                nc.vector.tensor_copy(
                    out=wtc[:, kb, :].rearrange("k (a j) -> k j a", a=4),
                    in_=ptw.rearrange("k (j a) -> k j a", a=4))
            # local wcol partial: sum over this core's 128 pooled features
            nc.vector.reduce_sum(
                out=stats[:, 24:32].rearrange("p (n o) -> p n o", o=1),
                in_=wtc, axis=mybir.AxisListType.X)

            cc_w_in = dram.tile([128, KF], BF16, space="DRAM")
            cc_w_out = dram.tile([N_CORES * 128, KF], BF16, space="DRAM")
            nc.gpsimd.dma_start(out=cc_w_in, in_=wtc)
            nc.gpsimd.collective_compute(
                "AllGather", mybir.AluOpType.bypass,
                replica_groups=[list(range(N_CORES))],
                ins=[cc_w_in.opt()], outs=[cc_w_out.opt()])

            # bias load (SWDGE, reuses the W slot once pooled).  Stamped
            # late so the AllGather doorbell leads the gpsimd queue.
            bload = wload.tile([1, NF], F32, tag="wl", name="bload")
            with tc.tile_wait_until(0.03):
                nc.gpsimd.dma_start(out=bload, in_=b)

            # ---- helpers ----
            ys_tiles = {}
            xT_tiles = {}

            def pool_tile(bt):
                ys = ys_tiles[bt]
                for nh in range(2):
                    yt = yts.pop((bt, nh))
                    ytv = yt.rearrange("p (q s) -> p q s", s=2)
                    u = yup.tile([128, KF], F32, tag="u", name=f"u{bt}_{nh}")
                    ueng = nc.vector if nh == 0 else nc.gpsimd
                    ueng.tensor_add(u, ytv[:, :, 0], ytv[:, :, 1])
                    u2 = u.rearrange("p (i r j) -> p i r j", r=2, j=32)
                    c = 2 * bt + nh
                    nc.vector.tensor_add(
                        ys[:, ts(nh, 512)].rearrange("p (i j) -> p i j", j=32),
                        u2[:, :, 0, :], u2[:, :, 1, :])
                    nc.vector.reduce_sum(
                        out=stats[:, c:c + 1],
                        in_=ys[:, ts(nh, 512)], axis=mybir.AxisListType.X)

            def xtrans(bt):
                xT = xtp.tile([128, 8, 128], BF16, tag=f"xT{bt}",
                              name=f"xT{bt}")
                for kb in range(8):
                    pt = psT.tile([128, 128], F32, tag="pt",
                                  name=f"ptx{bt}_{kb}")
                    nc.tensor.transpose(pt, xfs[bt][:, ts(kb, 128)], ident_f)
                    nc.scalar.copy(out=xT[:, kb, :], in_=pt)
                xT_tiles[bt] = xT

            def xsred(k):
                xs_r = statsp.tile([128, 8, 1], F32, tag="xs_r", bufs=2,
                                   name=f"xs_r{k}")
                nc.vector.reduce_sum(out=xs_r, in_=xT_tiles[k],
                                     axis=mybir.AxisListType.X)
                if k == 0:
                    nc.vector.tensor_copy(out=stats[:, 16:24],
                                          in_=xs_r[:, :, 0])
                else:
                    nc.vector.tensor_add(stats[:, 16:24], stats[:, 16:24],
                                         xs_r[:, :, 0])

            for bt in range(8):
                ys_tiles[bt] = ysump.tile([128, M], F32, tag=f"ys{bt}",
                                          name=f"ys{bt}")

            # ---- bias pooling (feeds the bias matmul + the payload) ----
            tc.tile_set_cur_wait(0.03)
            bsum = consts.tile([1, 32, 32], F32)
            bsum_bf = consts.tile([1, M], BF16)
            btot = consts.tile([1, 1], F32)
            btot_s = consts.tile([1, 1], F32)
            blv = bload.rearrange("o (i r j s) -> o i r j s", r=2, j=32, s=2)
            nc.vector.tensor_add(bsum, blv[:, :, 0, :, 0], blv[:, :, 0, :, 1])
            nc.vector.tensor_add(bsum, bsum, blv[:, :, 1, :, 0])
            nc.vector.tensor_add(bsum, bsum, blv[:, :, 1, :, 1])
            nc.vector.tensor_copy(
                out=bsum_bf, in_=bsum.rearrange("o i j -> o (i j)"))
            nc.vector.reduce_sum(
                out=btot, in_=bsum.rearrange("o i j -> o (i j)"),
                axis=mybir.AxisListType.X)
            nc.vector.tensor_scalar_mul(btot_s, btot, float(BL))

            # ---- gathered W readback (scalar ring) ----
            wt8 = wtp.tile([128, 8, 8, 128], BF16)
            for r in range(8):
                nc.scalar.dma_start(out=wt8[:, r],
                                    in_=cc_w_out[ts(r, 128), :])

            # ---- main loop: y stream + pooling + x transposes + stats.
            # Compute is stamped 0.03 so the AllGather doorbell (virtual
            # ~24us) is never queued behind it on any engine. ----
            for bt in range(8):
                if bt >= 2:
                    load_y(bt)
                with tc.tile_wait_until(0.03):
                    pool_tile(bt)
                    xtrans(bt)
                    xsred(bt)

            # fold the bias moment into the payload (after all col-0 writes)
            nc.vector.tensor_add(stats[0:1, 0:1], stats[0:1, 0:1], btot_s)

            # ---- AllReduce: fires at y-stream end, AllGather-independent ----
            cc_ar_in = dram.tile([128, 32], F32, space="DRAM")
            cc_ar_out = dram.tile([128, 32], F32, space="DRAM")
            nc.sync.dma_start(out=cc_ar_in, in_=stats)
            nc.gpsimd.collective_compute(
                "AllReduce", ADD,
                replica_groups=[list(range(N_CORES))],
                ins=[cc_ar_in.opt()], outs=[cc_ar_out.opt()])

            # ---- GEMM halves: psum added straight into ys by the DVE.
            # Stamps keep these AFTER the pool/stats chain in the DVE queue
            # so a late AllGather can never stall the AllReduce path. ----
            with tc.tile_wait_until(0.30):
                for bt in range(8):
                    for mh in range(2):
                        mm = psA.tile([128, 512], F32, tag="mm",
                                      name=f"mm{bt}_{mh}")
                        for kb in range(8):
                            nc.tensor.matmul(
                                mm, xT_tiles[bt][:, kb, :],
                                wt8[:, 4 * mh:4 * mh + 4, kb, :],
                                start=(kb == 0), stop=False)
                        nc.tensor.matmul(mm, ones_row,
                                         bsum_bf[:, ts(mh, 512)],
                                         start=False, stop=True)
                        nc.vector.tensor_add(ys_tiles[bt][:, ts(mh, 512)],
                                             ys_tiles[bt][:, ts(mh, 512)],
                                             mm)

            # ---- post-AR: gsum = sum(stats cols 0-15) + xsum_g . wcol_g ----
            with tc.tile_wait_until(0.32):
                S = statsp.tile([128, 32], F32)
                nc.scalar.dma_start(out=S, in_=cc_ar_out)
            with tc.tile_wait_until(0.33):
                tmp8 = statsp.tile([128, 8], F32)
                nc.vector.tensor_mul(tmp8, S[:, 16:24], S[:, 24:32])
                red1 = statsp.tile([128, 1], F32)
                red2 = statsp.tile([128, 1], F32)
                nc.vector.reduce_sum(out=red1, in_=S[:, 0:16],
                                     axis=mybir.AxisListType.X)
                nc.vector.reduce_sum(out=red2, in_=tmp8,
                                     axis=mybir.AxisListType.X)
                pcol2 = statsp.tile([128, 1], F32)
                nc.vector.tensor_add(pcol2, red1, red2)
                # partition-sum + broadcast on the (idle) PE
                g1ps = psT.tile([128, 128], F32, tag="pt", name="g1ps")
                nc.tensor.matmul(g1ps[0:1, 0:1], ones_col_f, pcol2,
                                 start=True, stop=True)
                g1sb = statsp.tile([1, 1], F32)
                nc.scalar.copy(out=g1sb, in_=g1ps[0:1, 0:1])
                gbps = psT.tile([128, 128], F32, tag="pt", name="gbps")
                nc.tensor.matmul(gbps[:, 0:1], ones_row_f, g1sb,
                                 start=True, stop=True)
                rsb = statsp.tile([128, 1], F32)
                nc.vector.reciprocal(rsb, gbps[:, 0:1])

            # ---- normalize + store, per-tile pipelined ----
            with tc.tile_wait_until(0.34):
                for bt in range(8):
                    meng = nc.vector if bt % 2 == 0 else nc.gpsimd
                    ot = outp.tile([128, M], F32, tag="ot", name=f"ot{bt}")
                    meng.tensor_scalar(out=ot, in0=ys_tiles[bt],
                                       scalar1=rsb, scalar2=TOT,
                                       op0=MULT, op1=MULT)
                    reng = nc.sync if bt % 2 == 0 else nc.scalar
                    reng.dma_start(out=out[ts(bt, 128), :], in_=ot)

    nc.compile()
    return nc


def _run(inputs, trace=False):
    if "nc" not in _CACHE:
        _CACHE["nc"] = build_nc()
    nc = _CACHE["nc"]
    x = np.ascontiguousarray(np.asarray(inputs["x"], dtype=np.float32))
    y = np.ascontiguousarray(np.asarray(inputs["y"], dtype=np.float32))
    w = np.ascontiguousarray(np.asarray(inputs["weight"], dtype=np.float32))
    b = np.ascontiguousarray(
        np.asarray(inputs["bias"], dtype=np.float32).reshape(1, NF))
    in_maps = [
        {"x": x[c * BL:(c + 1) * BL], "y": y[c * BL:(c + 1) * BL],
         "w": np.ascontiguousarray(w[c * WL:(c + 1) * WL]), "b": b}
        for c in range(N_CORES)
    ]
    res = run_bass_kernel_spmd(nc, in_maps, core_ids=list(range(N_CORES)),
                               trace=trace)
    full = np.concatenate([res.results[c]["out"] for c in range(N_CORES)],
                          axis=0)
    return full.reshape(B, 1, 32, 32), res


def kernel(**inputs) -> np.ndarray:
    out, _ = _run(inputs, trace=False)
    return out


# revision 17
# speedup vs baseline: 1.6421x; 1.1989x over previous
"""Fused GEMM + bias + residual + AvgPool2d(2) + global-mean normalize, 8-core SPMD.

Reference computation (B=8192, IN_F=1024, OUT_F=4096, S=64, K=2):
    out_lin = x @ W.T + bias + y                  # (B, 4096)
    pooled  = avgpool2x2(out_lin.reshape(B,64,64))# (B, 32, 32)
    out     = pooled / pooled.mean()              # (B, 1, 32, 32)

Key algebraic folds (all exact):
  * The 2x2 avg-pool folds into the weight/bias/residual:
        pooled_raw[b, m] = x[b] . Wsum[m] + bias_sum[m] + y_sum[b, m]
    (m = 32*i + j pools OUT_F rows {128i+2j, 128i+2j+1, 128i+64+2j,
    128i+64+2j+1}; GEMM N-dim shrinks 4096 -> 1024).
  * The 1/4 pool factor cancels: out = pooled_raw * (B*1024 / gsum).
  * gsum = xsum_g . wcol_g + B*bias_tot + ytot_g.  The AllReduce payload
    carries the xsum and wcol VECTORS alongside the y-rowsum stats and the
    dot is computed after the AllReduce, so the AllReduce depends only on
    local data -- never on the AllGather or the GEMM.

Distribution: batch split 8 ways; W sharded by row-block (core c loads rows
[512c, 512c+512), 2 MiB), pools to its 128 features, AllGathers the bf16
pooled shard k-major (so the PE does no W transposes post-gather).

Schedule (v8).  Trace lessons from v2/v6/v7:
  * The ncfw mesh AllGather has a FIXED latency profile: ~8us handshake,
    then the local 7-peer broadcast drains at the fold_n-limited ~50GB/s
    regardless of model-DMA contention, then a recv/event tail.  A 256KB
    shard costs ~80us trigger-to-done.  Starving the model DMA queues to
    "help" it does nothing (v7: +55us).  The only lever is payload size:
    two half-shards pipeline their sends, so the FIRST half lands ~35us
    earlier and the second about where the single one did.
  * GEMM is split into k-halves A (k 0..511) and B (k 512..1023): G_A runs
    as soon as AllGather A lands, G_B (plus the bias row) when B lands.
    Each half's psum is added straight into ys by the DVE (no ACT drain,
    no bf16 bounce) -- psA's 4 banks recycle against the DVE adds.
  * The DVE queue order is pinned with wait_until stamps so the y-pool /
    stats / AllReduce-trigger chain NEVER sits behind a GEMM-gated add:
    pools+stats (default) < addA (0.11) < addB (0.12) < post-AR (0.135)
    < normalize (0.14).  The AllReduce readback is stamped 0.13 so it
    cannot head-of-line-block readback B on the scalar ring.
  * Rings: sync = wl half, y stream, AR payload, even stores.
    scalar = wl half, x0-7, readbacks A+B, AR readback, odd stores.
"""

import numpy as np

import concourse.bass as bass
import concourse.mybir as mybir
import concourse.tile as tile
from concourse import bacc
from concourse.bass import ts
from concourse.bass_utils import run_bass_kernel_spmd
from concourse.masks import make_identity

N_CORES = 8
B = 8192
BL = B // N_CORES          # 1024 batch rows per core
KF = 1024                  # IN_F (contraction)
NF = 4096                  # OUT_F
WL = NF // N_CORES         # 512 W rows per core
M = 1024                   # pooled features (32*32)
TOT = float(B * M)         # elements in the global mean
F32 = mybir.dt.float32
BF16 = mybir.dt.bfloat16
ADD = mybir.AluOpType.add
MULT = mybir.AluOpType.mult

_CACHE = {}


def build_nc():
    nc = bacc.Bacc("TRN2", target_bir_lowering=False, debug=False,
                   num_devices=N_CORES)
    x = nc.dram_tensor("x", [BL, KF], F32, kind="ExternalInput").ap()
    y = nc.dram_tensor("y", [BL, NF], F32, kind="ExternalInput").ap()
    w = nc.dram_tensor("w", [WL, KF], F32, kind="ExternalInput").ap()
    b = nc.dram_tensor("b", [1, NF], F32, kind="ExternalInput").ap()
    out = nc.dram_tensor("out", [BL, M], F32, kind="ExternalOutput").ap()

    # This core's W rows n = 128a + 64r + 2j + s pool to local feature
    # m_local = 32a + j; (r, s) are the pool taps.  j-major load keeps DMA
    # descriptors wide; partition p = 4j + a.
    w_pairs = w.rearrange("(n s) k -> n (s k)", s=2)          # [256, 2048]
    wv = w_pairs.rearrange("(a r j) kk -> r j a kk", a=4, r=2, j=32)

    with tile.TileContext(nc) as tc:
        with (
            tc.tile_pool(name="consts", bufs=1) as consts,
            tc.tile_pool(name="wload", bufs=1) as wload,
            tc.tile_pool(name="wtmp", bufs=1) as wtmp,
            tc.tile_pool(name="wtp", bufs=1) as wtp,
            tc.tile_pool(name="xload", bufs=8) as xload,
            tc.tile_pool(name="xtp", bufs=1) as xtp,
            tc.tile_pool(name="yload", bufs=4) as yload,
            tc.tile_pool(name="yup", bufs=3) as yup,
            tc.tile_pool(name="ysump", bufs=1) as ysump,
            tc.tile_pool(name="statsp", bufs=1) as statsp,
            tc.tile_pool(name="outp", bufs=3) as outp,
            tc.tile_pool(name="psA", bufs=4, space="PSUM") as psA,
            tc.tile_pool(name="psT", bufs=3, space="PSUM") as psT,
            tc.tile_pool(name="dram", bufs=1, space="DRAM") as dram,
        ):
            # ---- constants ----
            ident_f = consts.tile([128, 128], F32)
            make_identity(nc, ident_f)
            ident_b = consts.tile([128, 128], BF16)
            make_identity(nc, ident_b)
            ones_row = consts.tile([1, 128], BF16)
            nc.vector.memset(ones_row, 1.0)
            ones_col_f = consts.tile([128, 1], F32)
            nc.vector.memset(ones_col_f, 1.0)
            ones_row_f = consts.tile([1, 128], F32)
            nc.vector.memset(ones_row_f, 1.0)

            # ---- queue heads ----
            wl = wload.tile([128, 2, 2048], F32, tag="wl", name="wl")
            nc.scalar.dma_start(out=wl[:, 0, :], in_=wv[0])
            nc.sync.dma_start(out=wl[:, 1, :], in_=wv[1])
            wlv = wl.rearrange("p r (s k) -> p r s k", s=2)

            xfs = []
            for bt in range(8):
                xf = xload.tile([128, KF], F32, tag="xf", name=f"xf{bt}")
                nc.scalar.dma_start(out=xf, in_=x[ts(bt, 128), :])
                xfs.append(xf)

            yts = {}

            def load_y(bt):
                for nh in range(2):
                    yt = yload.tile([128, 2048], F32, tag="yt",
                                    name=f"yt{bt}_{nh}")
                    nc.sync.dma_start(out=yt, in_=y[ts(bt, 128), ts(nh, 2048)])
                    yts[(bt, nh)] = yt

            load_y(0)
            load_y(1)

            # ---- stats: the AllReduce payload.  cols 0-15 y-pool rowsums,
            # 16-23 xsum (per k), 24-31 wcol partial (per k). ----
            stats = statsp.tile([128, 32], F32)

            # ---- W pooling + k-major transpose + 2 half AllGathers ----
            t1 = wtmp.tile([128, KF], F32)
            t2 = wtmp.tile([128, KF], F32)
            wsum = wtmp.tile([128, KF], BF16)
            wtc = wtmp.tile([128, 8, 128], BF16)
            nc.vector.tensor_add(t1, wlv[:, 0, 0], wlv[:, 0, 1])
            nc.vector.tensor_add(t2, wlv[:, 1, 0], wlv[:, 1, 1])
            nc.vector.tensor_add(wsum, t1, t2)
            for kb in range(8):
                ptw = psT.tile([128, 128], BF16, tag="pt", name=f"ptw{kb}")
                nc.tensor.transpose(ptw, wsum[:, ts(kb, 128)], ident_b)
                # undo the j-major load permutation p=4j+a -> 32a+j
                nc.vector.tensor_copy(
                    out=wtc[:, kb, :].rearrange("k (a j) -> k j a", a=4),
                    in_=ptw.rearrange("k (j a) -> k j a", a=4))
            # local wcol partial: sum over this core's 128 pooled features
            nc.vector.reduce_sum(
                out=stats[:, 24:32].rearrange("p (n o) -> p n o", o=1),
                in_=wtc, axis=mybir.AxisListType.X)

            cc_w_in = dram.tile([128, KF], BF16, space="DRAM")
            cc_w_out = dram.tile([N_CORES * 128, KF], BF16, space="DRAM")
            nc.gpsimd.dma_start(out=cc_w_in, in_=wtc)
            nc.gpsimd.collective_compute(
                "AllGather", mybir.AluOpType.bypass,
                replica_groups=[list(range(N_CORES))],
                ins=[cc_w_in.opt()], outs=[cc_w_out.opt()])

            # bias load (SWDGE, reuses the W slot once pooled).  Stamped
            # late so the AllGather doorbell leads the gpsimd queue.
            bload = wload.tile([1, NF], F32, tag="wl", name="bload")
            with tc.tile_wait_until(0.03):
                nc.gpsimd.dma_start(out=bload, in_=b)

            # ---- helpers ----
            ys_tiles = {}
            xT_tiles = {}

            def pool_tile(bt):
                ys = ys_tiles[bt]
                for nh in range(2):
                    yt = yts.pop((bt, nh))
                    ytv = yt.rearrange("p (q s) -> p q s", s=2)
                    u = yup.tile([128, KF], F32, tag="u", name=f"u{bt}_{nh}")
                    ueng = nc.vector if nh == 0 else nc.gpsimd
                    ueng.tensor_add(u, ytv[:, :, 0], ytv[:, :, 1])
                    u2 = u.rearrange("p (i r j) -> p i r j", r=2, j=32)
                    c = 2 * bt + nh
                    nc.vector.tensor_add(
                        ys[:, ts(nh, 512)].rearrange("p (i j) -> p i j", j=32),
                        u2[:, :, 0, :], u2[:, :, 1, :])
                    nc.vector.reduce_sum(
                        out=stats[:, c:c + 1],
                        in_=ys[:, ts(nh, 512)], axis=mybir.AxisListType.X)

            def xtrans(bt):
                xT = xtp.tile([128, 8, 128], BF16, tag=f"xT{bt}",
                              name=f"xT{bt}")
                for kb in range(8):
                    pt = psT.tile([128, 128], F32, tag="pt",
                                  name=f"ptx{bt}_{kb}")
                    nc.tensor.transpose(pt, xfs[bt][:, ts(kb, 128)], ident_f)
                    nc.scalar.copy(out=xT[:, kb, :], in_=pt)
                xT_tiles[bt] = xT

            def xsred(k):
                xs_r = statsp.tile([128, 8, 1], F32, tag="xs_r", bufs=2,
                                   name=f"xs_r{k}")
                nc.vector.reduce_sum(out=xs_r, in_=xT_tiles[k],
                                     axis=mybir.AxisListType.X)
                if k == 0:
                    nc.vector.tensor_copy(out=stats[:, 16:24],
                                          in_=xs_r[:, :, 0])
                else:
                    nc.vector.tensor_add(stats[:, 16:24], stats[:, 16:24],
                                         xs_r[:, :, 0])

            for bt in range(8):
                ys_tiles[bt] = ysump.tile([128, M], F32, tag=f"ys{bt}",
                                          name=f"ys{bt}")

            # ---- bias pooling (feeds the bias matmul + the payload) ----
            tc.tile_set_cur_wait(0.03)
            bsum = consts.tile([1, 32, 32], F32)
            bsum_bf = consts.tile([1, M], BF16)
            btot = consts.tile([1, 1], F32)
            btot_s = consts.tile([1, 1], F32)
            blv = bload.rearrange("o (i r j s) -> o i r j s", r=2, j=32, s=2)
            nc.vector.tensor_add(bsum, blv[:, :, 0, :, 0], blv[:, :, 0, :, 1])
            nc.vector.tensor_add(bsum, bsum, blv[:, :, 1, :, 0])
            nc.vector.tensor_add(bsum, bsum, blv[:, :, 1, :, 1])
            nc.vector.tensor_copy(
                out=bsum_bf, in_=bsum.rearrange("o i j -> o (i j)"))
            nc.vector.reduce_sum(
                out=btot, in_=bsum.rearrange("o i j -> o (i j)"),
                axis=mybir.AxisListType.X)
            nc.vector.tensor_scalar_mul(btot_s, btot, float(BL))

            # ---- gathered W readback (scalar ring).  Stamped past every
            # model DMA's virtual time: the completion-lane semaphores are
            # monotonic, so an AG-gated DMA placed mid-sequence transitively
            # stalls every later DMA on its lane. ----
            wt8 = wtp.tile([128, 8, 8, 128], BF16)
            with tc.tile_wait_until(0.25):
                for r in range(8):
                    nc.scalar.dma_start(out=wt8[:, r],
                                        in_=cc_w_out[ts(r, 128), :])

            # ---- main loop: y stream + pooling + x transposes + stats.
            # Compute is stamped 0.03 so the AllGather doorbell (virtual
            # ~24us) is never queued behind it on any engine. ----
            for bt in range(8):
                if bt >= 2:
                    load_y(bt)
                with tc.tile_wait_until(0.03):
                    pool_tile(bt)
                    xtrans(bt)
                    xsred(bt)

            # fold the bias moment into the payload (after all col-0 writes)
            nc.vector.tensor_add(stats[0:1, 0:1], stats[0:1, 0:1], btot_s)

            # ---- AllReduce: fires at y-stream end, AllGather-independent ----
            cc_ar_in = dram.tile([128, 32], F32, space="DRAM")
            cc_ar_out = dram.tile([128, 32], F32, space="DRAM")
            nc.sync.dma_start(out=cc_ar_in, in_=stats)
            nc.gpsimd.collective_compute(
                "AllReduce", ADD,
                replica_groups=[list(range(N_CORES))],
                ins=[cc_ar_in.opt()], outs=[cc_ar_out.opt()])

            # ---- GEMM halves: psum added straight into ys by the DVE.
            # Stamps keep these AFTER the pool/stats chain in the DVE queue
            # so a late AllGather can never stall the AllReduce path. ----
            with tc.tile_wait_until(0.30):
                for bt in range(8):
                    for mh in range(2):
                        mm = psA.tile([128, 512], F32, tag="mm",
                                      name=f"mm{bt}_{mh}")
                        for kb in range(8):
                            nc.tensor.matmul(
                                mm, xT_tiles[bt][:, kb, :],
                                wt8[:, 4 * mh:4 * mh + 4, kb, :],
                                start=(kb == 0), stop=False)
                        nc.tensor.matmul(mm, ones_row,
                                         bsum_bf[:, ts(mh, 512)],
                                         start=False, stop=True)
                        nc.vector.tensor_add(ys_tiles[bt][:, ts(mh, 512)],
                                             ys_tiles[bt][:, ts(mh, 512)],
                                             mm)

            # ---- post-AR: gsum = sum(stats cols 0-15) + xsum_g . wcol_g ----
            with tc.tile_wait_until(0.32):
                S = statsp.tile([128, 32], F32)
                nc.scalar.dma_start(out=S, in_=cc_ar_out)
            with tc.tile_wait_until(0.33):
                tmp8 = statsp.tile([128, 8], F32)
                nc.vector.tensor_mul(tmp8, S[:, 16:24], S[:, 24:32])
                red1 = statsp.tile([128, 1], F32)
                red2 = statsp.tile([128, 1], F32)
                nc.vector.reduce_sum(out=red1, in_=S[:, 0:16],
                                     axis=mybir.AxisListType.X)
                nc.vector.reduce_sum(out=red2, in_=tmp8,
                                     axis=mybir.AxisListType.X)
                pcol2 = statsp.tile([128, 1], F32)
                nc.vector.tensor_add(pcol2, red1, red2)
                # partition-sum + broadcast on the (idle) PE
                g1ps = psT.tile([128, 128], F32, tag="pt", name="g1ps")
                nc.tensor.matmul(g1ps[0:1, 0:1], ones_col_f, pcol2,
                                 start=True, stop=True)
                g1sb = statsp.tile([1, 1], F32)
                nc.scalar.copy(out=g1sb, in_=g1ps[0:1, 0:1])
                gbps = psT.tile([128, 128], F32, tag="pt", name="gbps")
                nc.tensor.matmul(gbps[:, 0:1], ones_row_f, g1sb,
                                 start=True, stop=True)
                rsb = statsp.tile([128, 1], F32)
                nc.vector.reciprocal(rsb, gbps[:, 0:1])

            # ---- normalize + store, per-tile pipelined ----
            with tc.tile_wait_until(0.34):
                for bt in range(8):
                    meng = nc.vector if bt % 2 == 0 else nc.gpsimd
                    ot = outp.tile([128, M], F32, tag="ot", name=f"ot{bt}")
                    meng.tensor_scalar(out=ot, in0=ys_tiles[bt],
                                       scalar1=rsb, scalar2=TOT,
                                       op0=MULT, op1=MULT)
                    reng = nc.sync if bt % 2 == 0 else nc.scalar
                    reng.dma_start(out=out[ts(bt, 128), :], in_=ot)

    nc.compile()
    return nc


def _run(inputs, trace=False):
    if "nc" not in _CACHE:
        _CACHE["nc"] = build_nc()
    nc = _CACHE["nc"]
    x = np.ascontiguousarray(np.asarray(inputs["x"], dtype=np.float32))
    y = np.ascontiguousarray(np.asarray(inputs["y"], dtype=np.float32))
    w = np.ascontiguousarray(np.asarray(inputs["weight"], dtype=np.float32))
    b = np.ascontiguousarray(
        np.asarray(inputs["bias"], dtype=np.float32).reshape(1, NF))
    in_maps = [
        {"x": x[c * BL:(c + 1) * BL], "y": y[c * BL:(c + 1) * BL],
         "w": np.ascontiguousarray(w[c * WL:(c + 1) * WL]), "b": b}
        for c in range(N_CORES)
    ]
    res = run_bass_kernel_spmd(nc, in_maps, core_ids=list(range(N_CORES)),
                               trace=trace)
    full = np.concatenate([res.results[c]["out"] for c in range(N_CORES)],
                          axis=0)
    return full.reshape(B, 1, 32, 32), res


def kernel(**inputs) -> np.ndarray:
    out, _ = _run(inputs, trace=False)
    return out
